# revision 18
# baseline (speedup 1.0000x reference)
"""Trainium2 Bass kernel for the Euler integrator with low-rank Christoffel
force — fp8 DoubleRow edition.

Reference semantics (per step, fp32):
    uv  = v @ U.T                      # [B,H]
    c   = (uv*uv) @ W.T                # [B,D]
    x  += dt*v   (uses OLD v)
    v  += dt*(force - c)
    x   = mod(x + pi, 2*pi) - pi

Strategy: data-parallel over 8 NeuronCores (batch 4096 -> 512 rows/core).
All matmuls run as fp8(e4m3) DoubleRow ("double pumping", 2 moving
rows/cycle — 2x the f32r streaming rate), with power-of-2 scale factors
chosen so every fp8 operand sits in e4m3's sweet spot (~N(0,1)):

    U8  = fp8(16*U.T)    packed [128, 2, H]   (d k-tile pair in dim 1)
    W8  = fp8(-32*W.T)   packed [128, 2, D] x 4 h-pairs
    vr  = fp8(v)         packed [128, 2, BS]

The velocity state never leaves PSUM: V = (32/dt)*v lives in one
[128, 2, BS] PSUM pair-bank for the whole kernel. Phase B accumulates
-32*c directly onto V (start=False), and a f32r identity matmul adds
32*force each step, so the per-step v update costs ZERO vector-engine
work. Read-outs:
    vr = fp8(V * dt/32)            one ACT Copy per step
    cx += (dt^2/32) * V            one DVE stt per step (old v, pre-update)
    sq = fp8(Square(uv_psum/16))   4 big [128,1024] ACT/DVE ops per step

uv PSUM tiles are [128, 2, BS] pairs (2 banks) so one activation squares
two h-tiles, and the result is already packed for the phase-B DoubleRow
moving operand.

Precision (validated by numpy e4m3 simulation + HW probe): rel err
x ~1.7e-2, v ~1.5e-2 vs the fp32 reference — inside the 2e-2 gate.
Errors are dominated by the one-time fp8 rounding of U and W; fp32
accumulation everywhere else (PSUM chains, x path).

The torus wrap follows the baseline proof: position is kept biased by
+pi and accumulated unwrapped; |x0 + pi| < ~8.6, |sum dt*v| < ~1.7 keep
it inside (-2pi, 4pi), where one final comparison-mask range reduction
into [0, 2pi) reproduces the reference's per-step mod exactly.
"""

import contextlib

import numpy as np
import ml_dtypes

import concourse.bacc as bacc
import concourse.mybir as mybir
import concourse.tile as tile
from concourse.bass_utils import run_bass_kernel_spmd

F32 = mybir.dt.float32
F32R = mybir.dt.float32r
F8 = mybir.dt.float8e4
NP8 = ml_dtypes.float8_e4m3
ALU = mybir.AluOpType
ACTF = mybir.ActivationFunctionType
DR = mybir.MatmulPerfMode.DoubleRow

N_CORES = 8
B = 4096
D = 256
H = 1024
P = 128
BS = B // N_CORES           # 512 batch rows per core
ND = D // P                 # 2 d partition-tiles
NH = H // P                 # 8 h partition-tiles
NPAIR = NH // 2             # 4 h-tile pairs

DT = np.float32(0.01 * 1.0)  # DT * DT_SCALE from the reference
PI = float(np.pi)
TWO_PI = float(2.0 * np.pi)

SU = np.float32(16.0)        # U pre-scale (16*U ~ N(0,1))
SW = np.float32(32.0)        # W pre-scale (32*W ~ N(0,1))
SV = np.float32(32.0 / DT)   # V state scale: V = SV * v

_PROGRAM_CACHE: dict = {}

# name -> (shape, numpy dtype) for bench.py's dummy input maps
BENCH_INPUTS = {
    "xpi": ([P, ND * BS], np.float32),
    "v0s": ([P, ND * BS], np.float32),
    "f32s": ([P, ND * BS], np.float32),
    "ut8": ([P, 2, H], NP8),
    "wt8": ([P, NPAIR, 2, D], NP8),
    "ident": ([P, P], np.float32),
}


def _build(steps: int, loop_reps: int | None = None, variant: str = "full",
           uv_bufs: int = 3, sq_bufs: int = 6,
           sq_eng: tuple = ("d", "a", "a", "a"), sq_cols_dve: int = 0,
           a_order: tuple = (1, 0, 2, 3), b_order: tuple = (1, 2, 0, 3),
           vr_mode: str = "act", f_early: bool = True,
           cx_from_vr: bool = True, cx_eng: str = "d"):
    # loop_reps: benchmarking only — wraps the step body in a hardware For_i
    # loop so device time scales well above wall-clock noise.
    # variant: "full" | "mm_only" (matmuls only, squares replaced by a
    # pre-DMAed dummy, no vr/cx read-outs) | "a_only" (phase A matmuls only)
    # sq_eng[jp]: 'a' (ACT square) or 'd' (DVE copy+mult) per uv pair
    # sq_cols_dve: batch-cols of each ACT pair delegated to DVE (col split)
    # a_order/b_order: pair emission order for phase A / consumption for B
    # vr_mode: 'act' | 'dve' | 'split' (ACT does d-half 0, DVE d-half 1)
    do_rw = variant == "full"   # read-outs (vr each step from V, cx stt)
    a_only = variant == "a_only"
    nc = bacc.Bacc(None, target_bir_lowering=False)

    x_d = nc.dram_tensor("xpi", [P, ND * BS], F32, kind="ExternalInput")
    v_d = nc.dram_tensor("v0s", [P, ND * BS], F32, kind="ExternalInput")
    f_d = nc.dram_tensor("f32s", [P, ND * BS], F32, kind="ExternalInput")
    u_d = nc.dram_tensor("ut8", [P, 2, H], F8, kind="ExternalInput")
    w_d = nc.dram_tensor("wt8", [P, NPAIR, 2, D], F8, kind="ExternalInput")
    i_d = nc.dram_tensor("ident", [P, P], F32, kind="ExternalInput")
    xo_d = nc.dram_tensor("xo", [P, ND * BS], F32, kind="ExternalOutput")
    vo_d = nc.dram_tensor("vo", [P, ND * BS], F32, kind="ExternalOutput")

    with tile.TileContext(nc) as tc:
        with (
            tc.tile_pool(name="state", bufs=1) as state,
            tc.tile_pool(name="sq", bufs=sq_bufs) as sqp,
            tc.tile_pool(name="vrp", bufs=2) as vrp,
            tc.tile_pool(name="tmp", bufs=3) as tmp,
            tc.tile_pool(name="psuv", bufs=uv_bufs, space="PSUM") as ps_uv,
            tc.tile_pool(name="psv", bufs=1, space="PSUM") as ps_v,
        ):
            ut8 = state.tile([P, 2, H], F8, name="ut8")
            wt8 = [state.tile([P, 2, D], F8, name=f"wt8{j}") for j in range(NPAIR)]
            cx = state.tile([P, ND * BS], F32, name="cx")
            v0s = state.tile([P, ND * BS], F32R, name="v0s")
            f32s = state.tile([P, ND * BS], F32R, name="f32s")
            ident = state.tile([P, P], F32R, name="ident")
            V = ps_v.tile([P, ND, BS], F32, name="V")

            # Input DMAs, first-needed-first, round-robined over 3 queues.
            xfers = [(v0s[:], v_d[:].bitcast(F32R)),
                     (ident[:], i_d[:].bitcast(F32R))]
            for j in range(NPAIR):
                xfers.append((ut8[:, :, j * D:(j + 1) * D],
                              u_d[:, :, j * D:(j + 1) * D]))
            for j in range(NPAIR):
                xfers.append((wt8[j][:], w_d[:, j, :, :]))
            xfers.append((f32s[:], f_d[:].bitcast(F32R)))
            xfers.append((cx[:], x_d[:]))
            queues = [nc.sync, nc.gpsimd, nc.scalar]
            for k, (dst, src) in enumerate(xfers):
                queues[k % len(queues)].dma_start(dst, src)

            # V init: identity matmuls load (32/dt)*v0 into the PSUM state.
            for i in range(ND):
                nc.tensor.matmul(V[:, i, :], ident[:], v0s[:, i * BS:(i + 1) * BS],
                                 start=True, stop=False)

            dummy_sq = None
            if not do_rw:
                dummy_sq = state.tile([P, 2, BS], F8, name="dsq")
                nc.sync.dma_start(dummy_sq[:], u_d[:, :, 0:BS])
                dummy_vr = state.tile([P, 2, BS], F8, name="dvr")
                nc.sync.dma_start(dummy_vr[:], u_d[:, :, 0:BS])

            def emit_step(last: bool):
                # -- read-outs of V (state of step t-1) first: vr (fp8
                # matmul operand) and the x integral (old v).
                if do_rw:
                    vr = vrp.tile([P, 2, BS], F8, tag="vr", name="vr")
                    sc = float(DT / 32.0)
                    if vr_mode == "act":
                        nc.scalar.activation(
                            vr[:], V[:].rearrange("p a b -> p (a b)"),
                            ACTF.Copy, scale=sc)
                    elif vr_mode == "dve":
                        nc.vector.tensor_scalar(
                            out=vr[:], in0=V[:].rearrange("p a b -> p (a b)"),
                            scalar1=sc, scalar2=None, op0=ALU.mult)
                    elif vr_mode == "split":  # ACT half 0, DVE half 1
                        nc.scalar.activation(
                            vr[:, 0, :], V[:, 0, :], ACTF.Copy, scale=sc)
                        nc.vector.tensor_scalar(
                            out=vr[:, 1, :], in0=V[:, 1, :],
                            scalar1=sc, scalar2=None, op0=ALU.mult)
                    else:  # split2: DVE takes half 0 (final B writes it
                        # first), ACT half 1 — both finish ~together
                        nc.vector.tensor_scalar(
                            out=vr[:, 0, :], in0=V[:, 0, :],
                            scalar1=sc, scalar2=None, op0=ALU.mult)
                        nc.scalar.activation(
                            vr[:, 1, :], V[:, 1, :], ACTF.Copy, scale=sc)
                    if not cx_from_vr:
                        nc.vector.scalar_tensor_tensor(
                            out=cx[:], in0=V[:].rearrange("p a b -> p (a b)"),
                            scalar=float(DT * DT / 32.0), in1=cx[:],
                            op0=ALU.mult, op1=ALU.add)
                else:
                    vr = dummy_vr

                # f32r identity adds 32*force onto V — emitted early so the
                # step's tail is the last B matmul, not the force add.
                def emit_f(stop):
                    for i in range(ND):
                        nc.tensor.matmul(
                            V[:, i, :], ident[:], f32s[:, i * BS:(i + 1) * BS],
                            start=False, stop=stop, skip_group_check=True)

                if not a_only and f_early:
                    emit_f(False)

                # -- phase A: uv pairs [128, 2, BS] PSUM, one DoubleRow
                # matmul per h-tile (K=256 in a single instruction).
                sq = {}
                for jp in a_order:
                    uvt = ps_uv.tile([P, 2, BS], F32, tag="uv", name="uv")
                    for i2 in range(2):
                        h0 = (2 * jp + i2) * P
                        nc.tensor.matmul(
                            uvt[:, i2, :], ut8[:, :, h0:h0 + P], vr[:],
                            start=True, stop=True, perf_mode=DR)
                    # squares: fp8(Square(uv/16)) packed for phase B
                    if do_rw:
                        sq_t = sqp.tile([P, 2, BS], F8, tag="sq", name="sq")
                        cd = BS if sq_eng[jp] == "d" else sq_cols_dve
                        if cd < BS:   # ACT part (leading cols)
                            ca = BS - cd
                            nc.scalar.activation(
                                sq_t[:, :, 0:ca], uvt[:, :, 0:ca],
                                ACTF.Square, scale=1.0 / 16.0)
                        if cd > 0:    # DVE part (trailing cols, copy+mult)
                            t = tmp.tile([P, 2, BS], F32, tag="t", name="t")
                            nc.vector.tensor_scalar(
                                out=t[:, :, 0:cd], in0=uvt[:, :, BS - cd:BS],
                                scalar1=1.0 / 16.0, scalar2=None, op0=ALU.mult)
                            nc.vector.tensor_tensor(
                                out=sq_t[:, :, BS - cd:BS],
                                in0=t[:, :, 0:cd], in1=t[:, :, 0:cd],
                                op=ALU.mult)
                        sq[jp] = sq_t
                    else:
                        sq[jp] = dummy_sq

                # late x integral off the critical chain: cx += dt*vr
                # (fp8 v copy; trades ~1e-3 x-accuracy for a V-free read)
                if do_rw and cx_from_vr:
                    eng = nc.gpsimd if cx_eng == "g" else nc.vector
                    eng.scalar_tensor_tensor(
                        out=cx[:], in0=vr[:].rearrange("p a b -> p (a b)"),
                        scalar=float(DT), in1=cx[:],
                        op0=ALU.mult, op1=ALU.add)

                # -- phase B: V += sum_jp W8[jp].T @ sq[jp]  (-32*c).
                # d-banks alternate so same-bank accumulation spacing is 2.
                if a_only:
                    return
                for jp in b_order:
                    for i in range(ND):
                        nc.tensor.matmul(
                            V[:, i, :], wt8[jp][:, :, i * P:(i + 1) * P],
                            sq[jp][:], start=False,
                            stop=(last and f_early and jp == b_order[-1]),
                            perf_mode=DR, skip_group_check=True)
                if not f_early:
                    emit_f(last)

            loop_cm = (
                tc.For_i(0, loop_reps, 1,
                         hint_engines=(mybir.EngineType.PE,
                                       mybir.EngineType.DVE,
                                       mybir.EngineType.Activation))
                if loop_reps is not None
                else contextlib.nullcontext()
            )
            with loop_cm:
                for s in range(steps):
                    emit_step(last=(s == steps - 1 and loop_reps is None))

            # final read-out + torus wrap into [0, 2pi)
            vo_sb = tmp.tile([P, ND * BS], F32, tag="vo", name="vo_sb")
            nc.scalar.activation(vo_sb[:], V[:].rearrange("p a b -> p (a b)"),
                                 ACTF.Copy, scale=float(DT / 32.0))
            g = tmp.tile([P, ND * BS], F32, tag="g", name="g")
            nc.vector.tensor_scalar(out=g[:], in0=cx[:], scalar1=TWO_PI,
                                    scalar2=None, op0=ALU.is_ge)
            lo = tmp.tile([P, ND * BS], F32, tag="l", name="l")
            nc.vector.tensor_scalar(out=lo[:], in0=cx[:], scalar1=0.0,
                                    scalar2=None, op0=ALU.is_lt)
            nc.vector.scalar_tensor_tensor(out=cx[:], in0=g[:], scalar=-TWO_PI,
                                           in1=cx[:], op0=ALU.mult, op1=ALU.add)
            nc.vector.scalar_tensor_tensor(out=cx[:], in0=lo[:], scalar=TWO_PI,
                                           in1=cx[:], op0=ALU.mult, op1=ALU.add)

            nc.sync.dma_start(xo_d[:], cx[:])
            nc.gpsimd.dma_start(vo_d[:], vo_sb[:])

    nc.compile()
    return nc


def _get_program(steps: int, loop_reps: int | None = None, variant: str = "full",
                 **kw):
    key = (steps, loop_reps, variant, tuple(sorted(kw.items())))
    if key not in _PROGRAM_CACHE:
        _PROGRAM_CACHE[key] = _build(steps, loop_reps, variant, **kw)
    return _PROGRAM_CACHE[key]


def _pack_dxb(a_bd, c):
    """[B, D] -> per-core [P, ND*BS]: halves along free dim are d-tiles."""
    at = a_bd.T[:, c * BS:(c + 1) * BS]              # [D, BS]
    return np.ascontiguousarray(
        at.reshape(ND, P, BS).transpose(1, 0, 2).reshape(P, ND * BS))


def _unpack_dxb(a, c, out):
    """inverse of _pack_dxb into out[B, D]."""
    at = a.reshape(P, ND, BS).transpose(1, 0, 2).reshape(D, BS)
    out[c * BS:(c + 1) * BS, :] = at.T


def _run(x, v, force, U, W, steps, trace=False):
    x = np.ascontiguousarray(np.asarray(x, dtype=np.float32))
    v = np.ascontiguousarray(np.asarray(v, dtype=np.float32))
    force = np.ascontiguousarray(np.asarray(force, dtype=np.float32))
    U = np.ascontiguousarray(np.asarray(U, dtype=np.float32))
    W = np.ascontiguousarray(np.asarray(W, dtype=np.float32))
    steps = int(np.asarray(steps).item()) if not isinstance(steps, int) else steps

    if steps == 0:
        # lax.scan with length 0 returns the carry untouched (no wrap)
        return (x.copy(), v.copy()), None

    nc = _get_program(steps)

    # fp8 weights, packed for DoubleRow stationary slices
    u8 = (SU * U).T.astype(NP8)                      # [D, H]
    ut8 = np.ascontiguousarray(
        u8.reshape(ND, P, H).transpose(1, 0, 2))     # [P, 2, H]
    w8 = (-SW * W).T.astype(NP8)                     # [H, D]
    wt8 = np.ascontiguousarray(
        w8.reshape(NPAIR, 2, P, D).transpose(2, 0, 1, 3))  # [P, 4, 2, D]
    ident = np.eye(P, dtype=np.float32)

    xpi_f = (x + np.float32(PI)).astype(np.float32)
    v0s_f = (SV * v).astype(np.float32)
    f32_f = (SW * force).astype(np.float32)

    in_maps = []
    for c in range(N_CORES):
        in_maps.append({
            "xpi": _pack_dxb(xpi_f, c),
            "v0s": _pack_dxb(v0s_f, c),
            "f32s": _pack_dxb(f32_f, c),
            "ut8": ut8,
            "wt8": wt8,
            "ident": ident,
        })

    try:
        res = run_bass_kernel_spmd(nc, in_maps, list(range(N_CORES)), trace=trace)
    except ModuleNotFoundError:
        # BASS_TRACE set in an env without the axon NTFF hook — retry untraced
        import os

        os.environ["BASS_NEVER_TRACE"] = "1"
        try:
            res = run_bass_kernel_spmd(nc, in_maps, list(range(N_CORES)))
        finally:
            os.environ.pop("BASS_NEVER_TRACE", None)

    xo = np.empty((B, D), dtype=np.float32)
    vo = np.empty((B, D), dtype=np.float32)
    for c in range(N_CORES):
        _unpack_dxb(res.results[c]["xo"], c, xo)
        _unpack_dxb(res.results[c]["vo"], c, vo)
    xo = (xo - np.float32(PI)).astype(np.float32)
    return (xo, vo), res


def kernel(x, v, force, U, W, steps):
    (xo, vo), _ = _run(x, v, force, U, W, steps)
    return xo, vo
